# revision 11
# baseline (speedup 1.0000x reference)
"""Trainium2 Bass kernel for LinearKG: y[n,o] = sum_r e[n,r]*(W[r]@x[n])[o] + (e@b)[n,o].

Strategy: single big matmul with contraction over (r,i) = 32*256 = 8192.
  z^T[(r,i), n] = e^T[r,n] * x^T[i,n]   (gpsimd partition-broadcast + DVE mul)
  y^T[o, n]     = sum_c W_c^T @ z^T_c + b_oh^T @ e^T   (PE, float32r, PSUM accum)
Data-parallel over nodes across 8 cores; W/b replicated. All transposes host-side.
"""

import numpy as np
import concourse.bass as bass
import concourse.bacc as bacc
import concourse.mybir as mybir
from concourse import tile
from concourse.bass_utils import run_bass_kernel_spmd

N, R, I, O = 100000, 32, 256, 256
NCORES = 8
NSUP = 512                 # nodes per super-tile (max f32 moving free dim)
SUPS = 25                  # supers per core
NS = NSUP * SUPS           # 12800 nodes per core
NP = NS * NCORES           # 102400 padded total
KC = R * I // 128          # 64 contraction chunks of 128

F32 = mybir.dt.float32
F32R = mybir.dt.float32r

_nc_cache = {}


def _build_nc():
    nc = bacc.Bacc("TRN2", target_bir_lowering=False, debug=False, num_devices=NCORES)
    xT = nc.declare_dram_parameter("xT", [I, NS], F32, isOutput=False)
    eT = nc.declare_dram_parameter("eT", [R, NS], F32R, isOutput=False)
    Wt = nc.declare_dram_parameter("Wt", [R, I, O], F32R, isOutput=False)  # W.transpose(0,2,1)
    bb = nc.declare_dram_parameter("b", [R, O], F32R, isOutput=False)
    yT = nc.declare_dram_parameter("yT", [O, NS], F32, isOutput=True)

    with tile.TileContext(nc) as tc:
        with (
            tc.tile_pool(name="wres", bufs=1) as wpool,
            tc.tile_pool(name="xin", bufs=3) as xpool,
            tc.tile_pool(name="ein", bufs=2) as epool,
            tc.tile_pool(name="ebc", bufs=3) as ebpool,
            tc.tile_pool(name="zt", bufs=6) as zpool,
            tc.tile_pool(name="yev", bufs=4) as ypool,
            tc.tile_pool(name="ps", bufs=4, space="PSUM") as pspool,
        ):
            # Resident weights: chunk c=(r*2+ih), col block (c*2+oh)*128.
            # lhsT chunk = Wt[r, ih*128:(ih+1)*128, oh*128:(oh+1)*128] -> [i_local, o_local]
            wt = wpool.tile([128, KC * 2 * 128], F32R)
            for r in range(R):
                for ih in range(2):
                    c = r * 2 + ih
                    for oh in range(2):
                        col = (c * 2 + oh) * 128
                        nc.sync.dma_start(
                            out=wt[:, col:col + 128],
                            in_=Wt[r, ih * 128:(ih + 1) * 128, oh * 128:(oh + 1) * 128],
                        )
            b_sb = wpool.tile([R, O], F32R)
            nc.sync.dma_start(out=b_sb[:], in_=bb[:, :])

            for s in range(SUPS):
                n0 = s * NSUP
                eT_t = epool.tile([R, NSUP], F32R)
                nc.sync.dma_start(out=eT_t[:], in_=eT[:, n0:n0 + NSUP])
                xT_t = xpool.tile([128, 2 * NSUP], F32)
                for ih in range(2):
                    nc.sync.dma_start(
                        out=xT_t[:, ih * NSUP:(ih + 1) * NSUP],
                        in_=xT[ih * 128:(ih + 1) * 128, n0:n0 + NSUP],
                    )
                ps0 = pspool.tile([128, NSUP], F32, tag="ps0")
                ps1 = pspool.tile([128, NSUP], F32, tag="ps1")
                ps = [ps0, ps1]
                GR = 8  # relations per broadcast group
                for g in range(R // GR):
                    # stage group rows on partition 0, then one wide broadcast
                    e_row = epool.tile([1, GR * NSUP], F32R, tag="e_row")
                    nc.sync.dma_start(
                        out=e_row[0:1, :].rearrange(
                            "p (r n) -> p r n", r=GR, n=NSUP
                        ),
                        in_=eT[g * GR:(g + 1) * GR, n0:n0 + NSUP].rearrange(
                            "(o r) n -> o r n", o=1
                        ),
                    )
                    eb = ebpool.tile([128, GR * NSUP], F32R)
                    nc.gpsimd.partition_broadcast(eb[:], e_row[0:1, :])
                    for rl in range(GR):
                        r = g * GR + rl
                        for ih in range(2):
                            c = r * 2 + ih
                            zt = zpool.tile([128, NSUP], F32R)
                            nc.vector.tensor_mul(
                                zt[:],
                                xT_t[:, ih * NSUP:(ih + 1) * NSUP],
                                eb[:, rl * NSUP:(rl + 1) * NSUP],
                            )
                            zr = zt[:]
                            for oh in range(2):
                                wcol = (c * 2 + oh) * 128
                                nc.tensor.matmul(
                                    ps[oh][:],
                                    lhsT=wt[:, wcol:wcol + 128],
                                    rhs=zr,
                                    start=(c == 0),
                                    stop=False,
                                )
                # bias: y^T[o_block, n] += b[:, o_block].T-style lhsT [R, 128] @ e^T [R, n]
                for oh in range(2):
                    nc.tensor.matmul(
                        ps[oh][:],
                        lhsT=b_sb[:, oh * 128:(oh + 1) * 128],
                        rhs=eT_t[:],
                        start=False,
                        stop=True,
                    )
                for oh in range(2):
                    yt = ypool.tile([128, NSUP], F32)
                    nc.scalar.copy(yt[:], ps[oh][:])
                    nc.sync.dma_start(
                        out=yT[oh * 128:(oh + 1) * 128, n0:n0 + NSUP], in_=yt[:]
                    )
    nc.compile()
    return nc


def _get_nc():
    if "nc" not in _nc_cache:
        _nc_cache["nc"] = _build_nc()
    return _nc_cache["nc"]


def kernel(x, edge_attr, W, b, _want_results_obj=False, _spmd_kwargs=None):
    x = np.asarray(x, dtype=np.float32)
    edge_attr = np.asarray(edge_attr, dtype=np.float32)
    W = np.asarray(W, dtype=np.float32)
    b = np.asarray(b, dtype=np.float32)

    nc = _get_nc()

    xT = np.zeros((I, NP), dtype=np.float32)
    xT[:, :N] = x.T
    eT = np.zeros((R, NP), dtype=np.float32)
    eT[:, :N] = edge_attr.T
    Wt = np.ascontiguousarray(W.transpose(0, 2, 1))

    in_maps = [
        {
            "xT": np.ascontiguousarray(xT[:, k * NS:(k + 1) * NS]),
            "eT": np.ascontiguousarray(eT[:, k * NS:(k + 1) * NS]),
            "Wt": Wt,
            "b": b,
        }
        for k in range(NCORES)
    ]
    res = run_bass_kernel_spmd(
        nc, in_maps, core_ids=list(range(NCORES)), **(_spmd_kwargs or {})
    )
    yT = np.concatenate([res.results[k]["yT"] for k in range(NCORES)], axis=1)
    y = np.ascontiguousarray(yT[:, :N].T)
    if _want_results_obj:
        return y, res
    return y


# revision 12
# speedup vs baseline: 1.1361x; 1.1361x over previous
"""Trainium2 Bass kernel for LinearKG: y[n,o] = sum_r e[n,r]*(W[r]@x[n])[o] + (e@b)[n,o].

Strategy: single big matmul with contraction over (r,i) = 32*256 = 8192.
  z^T[(r,i), n] = e^T[r,n] * x^T[i,n]   (gpsimd partition-broadcast + DVE mul)
  y^T[o, n]     = sum_c W_c^T @ z^T_c + b_oh^T @ e^T   (PE, float32r, PSUM accum)
Data-parallel over nodes across 8 cores; W/b replicated. All transposes host-side.
"""

import numpy as np
import concourse.bass as bass
import concourse.bacc as bacc
import concourse.mybir as mybir
from concourse import tile
from concourse.bass_utils import run_bass_kernel_spmd

N, R, I, O = 100000, 32, 256, 256
NCORES = 8
NSUP = 512                 # nodes per super-tile (max f32 moving free dim)
SUPS = 25                  # supers per core
NS = NSUP * SUPS           # 12800 nodes per core
NP = NS * NCORES           # 102400 padded total
KC = R * I // 128          # 64 contraction chunks of 128

F32 = mybir.dt.float32
F32R = mybir.dt.float32r
BF16 = mybir.dt.bfloat16

_nc_cache = {}


def _build_nc():
    nc = bacc.Bacc("TRN2", target_bir_lowering=False, debug=False, num_devices=NCORES)
    xT = nc.declare_dram_parameter("xT", [I, NS], BF16, isOutput=False)
    eT = nc.declare_dram_parameter("eT", [R, NS], BF16, isOutput=False)
    Wt = nc.declare_dram_parameter("Wt", [R, I, O], BF16, isOutput=False)  # W.transpose(0,2,1)
    bb = nc.declare_dram_parameter("b", [R, O], BF16, isOutput=False)
    yT = nc.declare_dram_parameter("yT", [O, NS], F32, isOutput=True)

    with tile.TileContext(nc) as tc:
        with (
            tc.tile_pool(name="wres", bufs=1) as wpool,
            tc.tile_pool(name="xin", bufs=3) as xpool,
            tc.tile_pool(name="ein", bufs=2) as epool,
            tc.tile_pool(name="ebc", bufs=3) as ebpool,
            tc.tile_pool(name="zt", bufs=6) as zpool,
            tc.tile_pool(name="yev", bufs=4) as ypool,
            tc.tile_pool(name="ps", bufs=4, space="PSUM") as pspool,
        ):
            # Resident weights: chunk c=(r*2+ih), col block (c*2+oh)*128.
            # lhsT chunk = Wt[r, ih*128:(ih+1)*128, oh*128:(oh+1)*128] -> [i_local, o_local]
            wt = wpool.tile([128, KC * 2 * 128], BF16)
            for r in range(R):
                for ih in range(2):
                    c = r * 2 + ih
                    for oh in range(2):
                        col = (c * 2 + oh) * 128
                        nc.sync.dma_start(
                            out=wt[:, col:col + 128],
                            in_=Wt[r, ih * 128:(ih + 1) * 128, oh * 128:(oh + 1) * 128],
                        )
            b_sb = wpool.tile([R, O], BF16)
            nc.sync.dma_start(out=b_sb[:], in_=bb[:, :])

            for s in range(SUPS):
                n0 = s * NSUP
                eT_t = epool.tile([R, NSUP], BF16)
                nc.sync.dma_start(out=eT_t[:], in_=eT[:, n0:n0 + NSUP])
                xT_t = xpool.tile([128, 2 * NSUP], BF16)
                for ih in range(2):
                    nc.sync.dma_start(
                        out=xT_t[:, ih * NSUP:(ih + 1) * NSUP],
                        in_=xT[ih * 128:(ih + 1) * 128, n0:n0 + NSUP],
                    )
                ps0 = pspool.tile([128, NSUP], F32, tag="ps0")
                ps1 = pspool.tile([128, NSUP], F32, tag="ps1")
                ps = [ps0, ps1]
                GR = 8  # relations per broadcast group
                for g in range(R // GR):
                    # stage group rows on partition 0, then one wide broadcast
                    e_row = epool.tile([1, GR * NSUP], BF16, tag="e_row")
                    nc.sync.dma_start(
                        out=e_row[0:1, :].rearrange(
                            "p (r n) -> p r n", r=GR, n=NSUP
                        ),
                        in_=eT[g * GR:(g + 1) * GR, n0:n0 + NSUP].rearrange(
                            "(o r) n -> o r n", o=1
                        ),
                    )
                    eb = ebpool.tile([128, GR * NSUP], BF16)
                    nc.gpsimd.partition_broadcast(eb[:], e_row[0:1, :])
                    for rl in range(GR):
                        r = g * GR + rl
                        for ih in range(2):
                            c = r * 2 + ih
                            zt = zpool.tile([128, NSUP], BF16)
                            nc.vector.tensor_mul(
                                zt[:],
                                xT_t[:, ih * NSUP:(ih + 1) * NSUP],
                                eb[:, rl * NSUP:(rl + 1) * NSUP],
                            )
                            zr = zt[:]
                            for oh in range(2):
                                wcol = (c * 2 + oh) * 128
                                nc.tensor.matmul(
                                    ps[oh][:],
                                    lhsT=wt[:, wcol:wcol + 128],
                                    rhs=zr,
                                    start=(c == 0),
                                    stop=False,
                                )
                # bias: y^T[o_block, n] += b[:, o_block].T-style lhsT [R, 128] @ e^T [R, n]
                for oh in range(2):
                    nc.tensor.matmul(
                        ps[oh][:],
                        lhsT=b_sb[:, oh * 128:(oh + 1) * 128],
                        rhs=eT_t[:],
                        start=False,
                        stop=True,
                    )
                for oh in range(2):
                    yt = ypool.tile([128, NSUP], F32)
                    nc.scalar.copy(yt[:], ps[oh][:])
                    nc.sync.dma_start(
                        out=yT[oh * 128:(oh + 1) * 128, n0:n0 + NSUP], in_=yt[:]
                    )
    nc.compile()
    return nc


def _get_nc():
    if "nc" not in _nc_cache:
        _nc_cache["nc"] = _build_nc()
    return _nc_cache["nc"]


def kernel(x, edge_attr, W, b, _want_results_obj=False, _spmd_kwargs=None):
    x = np.asarray(x, dtype=np.float32)
    edge_attr = np.asarray(edge_attr, dtype=np.float32)
    W = np.asarray(W, dtype=np.float32)
    b = np.asarray(b, dtype=np.float32)

    nc = _get_nc()

    import ml_dtypes
    bf16 = ml_dtypes.bfloat16
    xT = np.zeros((I, NP), dtype=bf16)
    xT[:, :N] = x.T.astype(bf16)
    eT = np.zeros((R, NP), dtype=bf16)
    eT[:, :N] = edge_attr.T.astype(bf16)
    Wt = np.ascontiguousarray(W.transpose(0, 2, 1)).astype(bf16)
    b = b.astype(bf16)

    in_maps = [
        {
            "xT": np.ascontiguousarray(xT[:, k * NS:(k + 1) * NS]),
            "eT": np.ascontiguousarray(eT[:, k * NS:(k + 1) * NS]),
            "Wt": Wt,
            "b": b,
        }
        for k in range(NCORES)
    ]
    res = run_bass_kernel_spmd(
        nc, in_maps, core_ids=list(range(NCORES)), **(_spmd_kwargs or {})
    )
    yT = np.concatenate([res.results[k]["yT"] for k in range(NCORES)], axis=1)
    y = np.ascontiguousarray(yT[:, :N].T)
    if _want_results_obj:
        return y, res
    return y


# revision 13
# speedup vs baseline: 1.2006x; 1.0568x over previous
"""Trainium2 Bass kernel for LinearKG: y[n,o] = sum_r e[n,r]*(W[r]@x[n])[o] + (e@b)[n,o].

Strategy: single big matmul with contraction over (r,i) = 32*256 = 8192.
  z^T[(r,i), n] = e^T[r,n] * x^T[i,n]   (gpsimd partition-broadcast + DVE mul)
  y^T[o, n]     = sum_c W_c^T @ z^T_c + b_oh^T @ e^T   (PE, float32r, PSUM accum)
Data-parallel over nodes across 8 cores; W/b replicated. All transposes host-side.
"""

import numpy as np
import concourse.bass as bass
import concourse.bacc as bacc
import concourse.mybir as mybir
from concourse import tile
from concourse.bass_utils import run_bass_kernel_spmd

N, R, I, O = 100000, 32, 256, 256
NCORES = 8
NSUP = 512                 # nodes per super-tile (max f32 moving free dim)
SUPS = 25                  # supers per core
NS = NSUP * SUPS           # 12800 nodes per core
NP = NS * NCORES           # 102400 padded total
KC = R * I // 128          # 64 contraction chunks of 128

F32 = mybir.dt.float32
F32R = mybir.dt.float32r
BF16 = mybir.dt.bfloat16

_nc_cache = {}


def _build_nc():
    nc = bacc.Bacc("TRN2", target_bir_lowering=False, debug=False, num_devices=NCORES)
    xT = nc.declare_dram_parameter("xT", [I, NS], BF16, isOutput=False)
    eT = nc.declare_dram_parameter("eT", [R, NS], BF16, isOutput=False)
    Wt = nc.declare_dram_parameter("Wt", [R, I, O], BF16, isOutput=False)  # W.transpose(0,2,1)
    bb = nc.declare_dram_parameter("b", [R, O], BF16, isOutput=False)
    yT = nc.declare_dram_parameter("yT", [O, NS], F32, isOutput=True)

    with tile.TileContext(nc) as tc:
        with (
            tc.tile_pool(name="wres", bufs=1) as wpool,
            tc.tile_pool(name="xin", bufs=3) as xpool,
            tc.tile_pool(name="ein", bufs=2) as epool,
            tc.tile_pool(name="ebc", bufs=2) as ebpool,
            tc.tile_pool(name="zt", bufs=8) as zpool,
            tc.tile_pool(name="yev", bufs=4) as ypool,
            tc.tile_pool(name="ps", bufs=4, space="PSUM") as pspool,
        ):
            # Resident weights: chunk c=(r*2+ih), col block (c*2+oh)*128.
            # lhsT chunk = Wt[r, ih*128:(ih+1)*128, oh*128:(oh+1)*128] -> [i_local, o_local]
            wt = wpool.tile([128, KC * 2 * 128], BF16)
            for r in range(R):
                for ih in range(2):
                    c = r * 2 + ih
                    for oh in range(2):
                        col = (c * 2 + oh) * 128
                        nc.sync.dma_start(
                            out=wt[:, col:col + 128],
                            in_=Wt[r, ih * 128:(ih + 1) * 128, oh * 128:(oh + 1) * 128],
                        )
            b_sb = wpool.tile([R, O], BF16)
            nc.sync.dma_start(out=b_sb[:], in_=bb[:, :])

            for s in range(SUPS):
                n0 = s * NSUP
                eT_t = epool.tile([R, NSUP], BF16)
                nc.sync.dma_start(out=eT_t[:], in_=eT[:, n0:n0 + NSUP])
                xT_t = xpool.tile([128, 2 * NSUP], BF16)
                for ih in range(2):
                    nc.sync.dma_start(
                        out=xT_t[:, ih * NSUP:(ih + 1) * NSUP],
                        in_=xT[ih * 128:(ih + 1) * 128, n0:n0 + NSUP],
                    )
                ps0 = pspool.tile([128, NSUP], F32, tag="ps0")
                ps1 = pspool.tile([128, NSUP], F32, tag="ps1")
                ps = [ps0, ps1]
                GR = 32  # relations per broadcast group
                for g in range(R // GR):
                    # stage group rows on partition 0, then one wide broadcast
                    e_row = epool.tile([1, GR * NSUP], BF16, tag="e_row")
                    nc.sync.dma_start(
                        out=e_row[0:1, :].rearrange(
                            "p (r n) -> p r n", r=GR, n=NSUP
                        ),
                        in_=eT[g * GR:(g + 1) * GR, n0:n0 + NSUP].rearrange(
                            "(o r) n -> o r n", o=1
                        ),
                    )
                    eb = ebpool.tile([128, GR * NSUP], BF16)
                    nc.gpsimd.partition_broadcast(eb[:], e_row[0:1, :])
                    for rl in range(GR):
                        r = g * GR + rl
                        for ih in range(2):
                            c = r * 2 + ih
                            zt = zpool.tile([128, NSUP], BF16)
                            nc.vector.tensor_mul(
                                zt[:],
                                xT_t[:, ih * NSUP:(ih + 1) * NSUP],
                                eb[:, rl * NSUP:(rl + 1) * NSUP],
                            )
                            zr = zt[:]
                            for oh in range(2):
                                wcol = (c * 2 + oh) * 128
                                nc.tensor.matmul(
                                    ps[oh][:],
                                    lhsT=wt[:, wcol:wcol + 128],
                                    rhs=zr,
                                    start=(c == 0),
                                    stop=False,
                                )
                # bias: y^T[o_block, n] += b[:, o_block].T-style lhsT [R, 128] @ e^T [R, n]
                for oh in range(2):
                    nc.tensor.matmul(
                        ps[oh][:],
                        lhsT=b_sb[:, oh * 128:(oh + 1) * 128],
                        rhs=eT_t[:],
                        start=False,
                        stop=True,
                    )
                for oh in range(2):
                    yt = ypool.tile([128, NSUP], F32)
                    nc.scalar.copy(yt[:], ps[oh][:])
                    nc.sync.dma_start(
                        out=yT[oh * 128:(oh + 1) * 128, n0:n0 + NSUP], in_=yt[:]
                    )
    nc.compile()
    return nc


def _get_nc():
    if "nc" not in _nc_cache:
        _nc_cache["nc"] = _build_nc()
    return _nc_cache["nc"]


def kernel(x, edge_attr, W, b, _want_results_obj=False, _spmd_kwargs=None):
    x = np.asarray(x, dtype=np.float32)
    edge_attr = np.asarray(edge_attr, dtype=np.float32)
    W = np.asarray(W, dtype=np.float32)
    b = np.asarray(b, dtype=np.float32)

    nc = _get_nc()

    import ml_dtypes
    bf16 = ml_dtypes.bfloat16
    xT = np.zeros((I, NP), dtype=bf16)
    xT[:, :N] = x.T.astype(bf16)
    eT = np.zeros((R, NP), dtype=bf16)
    eT[:, :N] = edge_attr.T.astype(bf16)
    Wt = np.ascontiguousarray(W.transpose(0, 2, 1)).astype(bf16)
    b = b.astype(bf16)

    in_maps = [
        {
            "xT": np.ascontiguousarray(xT[:, k * NS:(k + 1) * NS]),
            "eT": np.ascontiguousarray(eT[:, k * NS:(k + 1) * NS]),
            "Wt": Wt,
            "b": b,
        }
        for k in range(NCORES)
    ]
    res = run_bass_kernel_spmd(
        nc, in_maps, core_ids=list(range(NCORES)), **(_spmd_kwargs or {})
    )
    yT = np.concatenate([res.results[k]["yT"] for k in range(NCORES)], axis=1)
    y = np.ascontiguousarray(yT[:, :N].T)
    if _want_results_obj:
        return y, res
    return y
